# revision 26
# baseline (speedup 1.0000x reference)
"""GNN message-passing net on 8 Trainium2 cores.

Reference: x:[256,784,1] -> h1 = elu(spmm(x)@W1+b1) -> h2 = elu(spmm(h1)@W2+b2)
-> flat[B, N*C] -> relu(flat@Wf1+bf1) -> softmax(z@Wf2+bf2).

Strategy (all matmul operands bf16, fp32 PSUM accumulation):
  * Densify the sparse filter A (784x784, ~1% nz) on the host; spmm becomes
    dense matmuls on the PE array.
  * F=1 makes conv1 an outer product: out1 = A @ X^T [784,256] shared by all
    channels; h1_c = elu(W1[c]*out1+b1[c]) via the exact identity
    elu(t) = max(t, min(exp(t),1)-1): one ACT Exp pass + three DVE ops.
  * Conv2 spmm channel-sharded: core k computes out2_c = A @ h1_c for
    channels 4k..4k+3, in two batch halves with kc-OUTER accumulation (6
    PSUM accumulators + one kc-inner chain) so the first half consumes h1
    chunks as the conv1 elu pipeline produces them instead of waiting for
    all of h1.
  * One AllToAll reshards channel->node: core j gets all 32 pre-mix
    channels for nodes [112j, 112j+112) (core 7 gets zero-padded nodes
    784..895), packed [(ng,c), s, b].  Packs are spread across three DMA
    queues as each node block completes; the unpack is s-range-major so
    the W2 mix starts before the full payload has landed.
  * W2 channel mix as a 128x128 stationary kron(I4,W2) matmul; +b2, elu.
  * W2 mix runs as 4-node-group chains in two-bank PSUM tiles so the elu
    chain latency amortizes over 1024-wide tiles.
  * FC1 stays K-sharded: core k holds Wf1 rows for its nodes, 28 K-chunks x
    4 h-chunks, free=256.  z^T partials [512,256] are ReduceScattered in
    bf16 (halves the collective payload); each core then does +bf1, relu,
    FC2 (+bf2 via a ones-row matmul) and softmax for its 32-batch block.
  Note: each matmul accumulation chain needs its own PSUM bank — two
  start/stop chains sharing a bank corrupt each other.
"""
import json

import numpy as np

import concourse.bass as bass
import concourse.mybir as mybir
import concourse.tile as tile
from concourse.bass_utils import run_bass_kernel_spmd

B, N, F, E = 256, 784, 1, 6272
C, H, N_OUT = 32, 512, 10
NCORE = 8
CPC = C // NCORE      # 4 channels per core in conv2
P = 112               # 784 = 7 * 112
KN = N // P           # 7 node chunks
NPAD = P * NCORE      # 896 padded nodes for the node reshard
NG = 4                # node groups packed into partitions for the mix
NS = P // NG          # 28 nodes per group per core
BPC = B // NCORE      # 32 batch rows per core
HJ = H // 128         # 4 h chunks
BH = 2                # batch halves for the pipelined a2a
BB = B // BH          # 128 batch rows per half

f32 = mybir.dt.float32
bf16 = mybir.dt.bfloat16
AF = mybir.ActivationFunctionType
ALU = mybir.AluOpType
AX = mybir.AxisListType

GROUPS = [list(range(NCORE))]


# ---------------------------------------------------------------------------
# BIR post-pass: this walrus build rejects instructions with >1 sync-wait;
# split extras onto standalone EventSemaphore instructions (same engine,
# inserted just before, so the engine stream stalls identically).
def _split_waits(bir: dict, max_waits: int = 1) -> dict:
    n = [0]
    for fn in bir.get("functions", []):
        for blk in fn.get("blocks", []):
            out = []
            for ins in blk.get("instructions", []):
                si = ins.get("sync_info") or {}
                waits = si.get("on_wait") or []
                if len(waits) > max_waits:
                    for w in waits[max_waits:]:
                        n[0] += 1
                        out.append({
                            "name": f"I-waitsplit-{n[0]}",
                            "opcode": "EventSemaphore",
                            "engine": ins["engine"],
                            "ins": [], "outs": [],
                            **({"debug": ins["debug"]} if "debug" in ins else {}),
                            "sync_info": {"on_update": [], "on_wait": [w]},
                        })
                    si = dict(si)
                    si["on_wait"] = waits[:max_waits]
                    ins = dict(ins)
                    ins["sync_info"] = si
                out.append(ins)
            blk["instructions"] = out
    return bir


def _install_wait_splitter(nc):
    orig = nc.to_json_bytes
    nc.to_json_bytes = lambda: json.dumps(_split_waits(json.loads(orig()))).encode()


# ---------------------------------------------------------------------------
def _build_program():
    nc = bass.Bass(num_devices=NCORE)

    at_d = nc.dram_tensor("at", [P, KN * N], bf16, kind="ExternalInput")
    xt_d = nc.dram_tensor("xt", [P, KN * B], bf16, kind="ExternalInput")
    wf1_d = nc.dram_tensor("wf1", [NS * 128, H], bf16, kind="ExternalInput")
    wb_d = nc.dram_tensor("wb", [128, 2 * CPC], f32, kind="ExternalInput")
    w2k_d = nc.dram_tensor("w2k", [128, 128], bf16, kind="ExternalInput")
    b2k_d = nc.dram_tensor("b2k", [128, 1], f32, kind="ExternalInput")
    bf1_d = nc.dram_tensor("bf1", [128, HJ], f32, kind="ExternalInput")
    wf2_d = nc.dram_tensor("wf2", [128, HJ * N_OUT], bf16, kind="ExternalInput")
    bf2_d = nc.dram_tensor("bf2", [1, N_OUT], bf16, kind="ExternalInput")
    out_d = nc.dram_tensor("out", [BPC, N_OUT], f32, kind="ExternalOutput")

    with tile.TileContext(nc) as tc:
        with (
            tc.tile_pool(name="big", bufs=1) as big,
            tc.tile_pool(name="small", bufs=1) as small,
            tc.tile_pool(name="ework", bufs=2) as ework,
            tc.tile_pool(name="twork", bufs=2) as twork,
            tc.tile_pool(name="dram", bufs=1, space="DRAM") as dram,
        ):
            # ---- resident inputs ------------------------------------------
            at_sb = big.tile([P, KN, N], bf16)
            xt_sb = big.tile([P, KN, B], bf16)
            wf1_sb = big.tile([128, NS, H], bf16)
            wband = small.tile([128, 2 * CPC], f32)
            w2k_sb = small.tile([128, 128], bf16)
            b2k_sb = small.tile([128, 1], f32)
            bf1_sb = small.tile([128, HJ], f32)
            wf2_sb = small.tile([128, HJ, N_OUT], bf16)
            bf2_sb = small.tile([1, N_OUT], bf16)
            ones = small.tile([1, 128], bf16)
            zpad = small.tile([P, CPC, B], bf16)

            nc.sync.dma_start(wband[:], wb_d[:])
            at_ap = at_d[:].rearrange("p (k n) -> p k n", k=KN)
            xt_ap = xt_d[:].rearrange("p (k b) -> p k b", k=KN)
            for kc in range(KN):
                nc.sync.dma_start(xt_sb[:, kc, :], xt_ap[:, kc, :])
                nc.sync.dma_start(at_sb[:, kc, :], at_ap[:, kc, :])
            nc.sync.dma_start(w2k_sb[:], w2k_d[:])
            nc.sync.dma_start(b2k_sb[:], b2k_d[:])
            nc.sync.dma_start(bf1_sb[:], bf1_d[:])
            nc.sync.dma_start(wf2_sb[:], wf2_d[:].rearrange("p (j o) -> p j o",
                                                            j=HJ))
            nc.sync.dma_start(bf2_sb[:], bf2_d[:])
            ab_sb = small.tile([NCORE, 2], f32)
            nc.vector.memset(ab_sb[:], 0.0)
            nc.vector.memset(ones[:], 1.0)
            nc.vector.memset(zpad[:], 0.0)
            ab_d = dram.tile([NCORE, 2], f32)
            nc.sync.dma_start(ab_d[:], ab_sb[:])
            nc.sync.dma_start(wf1_sb[:],
                              wf1_d[:].rearrange("(s p) h -> p s h", p=128))
            # warm-up collective with no consumers: pays the first-collective
            # setup (~12us) and any launch skew on the CC engine while the
            # compute engines run conv1, so the real a2a starts promptly.
            # Same kind as the real collective so any AllToAll-specific ring
            # setup is prepaid too.
            ab_o = dram.tile([NCORE, 2], f32)
            nc.gpsimd.collective_compute(
                "AllToAll", ALU.bypass, replica_groups=GROUPS,
                ins=[ab_d.opt()], outs=[ab_o.opt()],
            )

            h1_sb = big.tile([P, CPC, KN, B], bf16)
            out2_sb = big.tile([P, CPC, KN, B], bf16)
            r_sb = big.tile([128, NS, B], bf16)
            h2_sb = big.tile([128, NS, B], bf16)

            a2a_in = dram.tile([NCORE, CPC * P, B], bf16)
            a2a_out = dram.tile([NCORE, CPC * P, B], bf16)

            with (
                tc.tile_pool(name="ps1", bufs=2, space="PSUM") as ps1,
                tc.tile_pool(name="ps2", bufs=KN - 1, space="PSUM") as ps2,
            ):
                # ---- conv1 + elu ------------------------------------------
                for mc in range(KN):
                    o1 = ps1.tile([P, B], f32, tag="ps1", name="o1")
                    for kc in range(KN):
                        nc.tensor.matmul(
                            o1[:],
                            at_sb[:, kc, mc * P:(mc + 1) * P],
                            xt_sb[:, kc, :],
                            start=(kc == 0), stop=(kc == KN - 1),
                        )
                    e4 = ework.tile([P, CPC, B], bf16, name="e4")
                    t4 = twork.tile([P, CPC, B], bf16, name="t4")
                    for c in range(CPC):
                        nc.scalar.activation(
                            e4[:, c, :], o1[:], AF.Exp,
                            bias=wband[0:P, CPC + c:CPC + c + 1],
                            scale=wband[0:P, c:c + 1])
                    for c in range(CPC):
                        if c == 3:
                            nc.scalar.activation(
                                t4[:, c, :], o1[:], AF.Identity,
                                bias=wband[0:P, CPC + c:CPC + c + 1],
                                scale=wband[0:P, c:c + 1])
                        else:
                            nc.vector.tensor_scalar(
                                t4[:, c, :], o1[:],
                                wband[0:P, c:c + 1],
                                wband[0:P, CPC + c:CPC + c + 1],
                                ALU.mult, ALU.add)
                    nc.vector.tensor_scalar(e4[:], e4[:], 1.0, -1.0,
                                            ALU.min, ALU.add)
                    nc.vector.tensor_tensor(h1_sb[:, :, mc, :], t4[:], e4[:],
                                            ALU.max)

                # ---- conv2 spmm: kc-outer per batch half (overlaps the
                # conv1 elu pipeline), 7 accumulators, single a2a ----------
                pack_engines = [nc.gpsimd, nc.sync, nc.scalar]
                for bh in range(BH):
                    bs = bh * BB
                    o2 = [ps2.tile([P, CPC, BB], f32, tag="ps2", name="o2")
                          for mo in range(KN - 1)]
                    for kc in range(KN):
                        for mo in range(KN - 1):
                            nc.tensor.matmul(
                                o2[mo][:],
                                at_sb[:, kc, mo * P:(mo + 1) * P],
                                h1_sb[:, :, kc, bs:bs + BB],
                                start=(kc == 0), stop=(kc == KN - 1),
                            )
                    for mo in range(KN - 1):
                        if mo % 2 == 0:
                            nc.scalar.copy(out2_sb[:, :, mo, bs:bs + BB],
                                           o2[mo][:])
                        else:
                            nc.vector.tensor_copy(
                                out2_sb[:, :, mo, bs:bs + BB], o2[mo][:])
                        dstp = a2a_in[mo].rearrange(
                            "(cl p) b -> p cl b", cl=CPC)[:, :, bs:bs + BB]
                        pack_engines[mo % 3].dma_start(
                            dstp, out2_sb[:, :, mo, bs:bs + BB])
                    # mo = KN-1: plain kc-inner chain (no spare PSUM bank)
                    o2l = ps2.tile([P, CPC, BB], f32, tag="ps2", name="o2")
                    for kc in range(KN):
                        nc.tensor.matmul(
                            o2l[:],
                            at_sb[:, kc, (KN - 1) * P:KN * P],
                            h1_sb[:, :, kc, bs:bs + BB],
                            start=(kc == 0), stop=(kc == KN - 1),
                        )
                    nc.scalar.copy(out2_sb[:, :, KN - 1, bs:bs + BB], o2l[:])
                    dstp = a2a_in[KN - 1].rearrange(
                        "(cl p) b -> p cl b", cl=CPC)[:, :, bs:bs + BB]
                    pack_engines[(KN - 1) % 3].dma_start(
                        dstp, out2_sb[:, :, KN - 1, bs:bs + BB])
                dstp = a2a_in[KN].rearrange("(cl p) b -> p cl b", cl=CPC)
                nc.gpsimd.dma_start(dstp, zpad[:])
                nc.gpsimd.collective_compute(
                    "AllToAll", ALU.bypass, replica_groups=GROUPS,
                    ins=[a2a_in.opt()], outs=[a2a_out.opt()],
                )
                # unpack s-range-major so the W2 mix can start before the
                # whole payload has landed; spread across DMA queues.
                ap = a2a_out[:].rearrange(
                    "k (cl g s) b -> g (k cl) s b", cl=CPC, g=NG)
                unpack_engines = [nc.sync, nc.scalar, nc.gpsimd]
                ui = 0
                for s0 in range(0, NS, 7):
                    for ng in range(NG):
                        unpack_engines[ui % 3].dma_start(
                            r_sb[ng * C:(ng + 1) * C, s0:s0 + 7, :],
                            ap[ng, :, s0:s0 + 7, :])
                        ui += 1

            with (
                tc.tile_pool(name="ps_mix", bufs=2, space="PSUM") as ps_mix,
                tc.tile_pool(name="ps_fc1", bufs=HJ, space="PSUM") as ps_fc1,
            ):
                # ---- W2 mix + b2 + elu -> h2 (bf16) -----------------------
                zps = [ps_fc1.tile([128, B], f32, tag="psfc1", name="zp")
                       for h in range(HJ)]
                for s0 in range(0, NS, 4):
                    pm = ps_mix.tile([128, 4, B], f32, tag="psm", name="pm")
                    nc.tensor.matmul(pm[:, 0:2, :], w2k_sb[:],
                                     r_sb[:, s0:s0 + 2, :])
                    nc.tensor.matmul(pm[:, 2:4, :], w2k_sb[:],
                                     r_sb[:, s0 + 2:s0 + 4, :])
                    em = ework.tile([128, 4, B], bf16, name="em")
                    tm = twork.tile([128, 4, B], bf16, name="tm")
                    nc.scalar.activation(em[:], pm[:], AF.Exp,
                                         bias=b2k_sb[:, 0:1])
                    if (s0 // 4) % 2 == 0:
                        nc.scalar.activation(tm[:], pm[:], AF.Identity,
                                             bias=b2k_sb[:, 0:1])
                    else:
                        nc.vector.tensor_scalar(tm[:], pm[:], b2k_sb[:, 0:1],
                                                None, ALU.add)
                    nc.vector.tensor_scalar(em[:], em[:], 1.0, -1.0,
                                            ALU.min, ALU.add)
                    nc.vector.tensor_tensor(h2_sb[:, s0:s0 + 4, :],
                                            tm[:], em[:], ALU.max)
                    # FC1 s-groups trail the mix by two chains so their
                    # h2 waits are already satisfied at issue time (a lag of
                    # zero head-of-line-blocks the next chain's matmuls).
                    if s0 >= 8:
                        for s in range(s0 - 8, s0 - 4):
                            for hj in range(HJ):
                                nc.tensor.matmul(
                                    zps[hj][:],
                                    wf1_sb[:, s, hj * 128:(hj + 1) * 128],
                                    h2_sb[:, s, :],
                                    start=(s == 0), stop=(s == NS - 1),
                                )
                for s in range(NS - 8, NS):
                    for hj in range(HJ):
                        nc.tensor.matmul(
                            zps[hj][:],
                            wf1_sb[:, s, hj * 128:(hj + 1) * 128],
                            h2_sb[:, s, :],
                            start=(s == 0), stop=(s == NS - 1),
                        )

                # ---- ReduceScatter z partials -----------------------------
                zsb = small.tile([128, HJ, B], bf16)
                for hj in range(HJ):
                    if hj % 2 == 0:
                        nc.scalar.copy(zsb[:, hj, :], zps[hj][:])
                    else:
                        nc.vector.tensor_copy(zsb[:, hj, :], zps[hj][:])
                rs_in = dram.tile([NCORE, H, BPC], bf16)
                rs_out = dram.tile([H, BPC], bf16)
                rdst = rs_in[:].rearrange("j h b -> h j b")
                rs_engines = [nc.sync, nc.scalar, nc.gpsimd, nc.sync]
                for hj in range(HJ):
                    rs_engines[hj].dma_start(
                        rdst[hj * 128:(hj + 1) * 128],
                        zsb[:, hj, :].rearrange("h (j b) -> h j b", j=NCORE),
                    )
                nc.gpsimd.collective_compute(
                    "ReduceScatter", ALU.add, replica_groups=GROUPS,
                    ins=[rs_in.opt()], outs=[rs_out.opt()],
                )

                # ---- +bf1, relu, FC2, +bf2, softmax -----------------------
                z_sb = small.tile([128, HJ, BPC], bf16)
                zr_sb = small.tile([128, HJ, BPC], bf16)
                z_ap = rs_out[:].rearrange("(hj p) b -> p hj b", p=128)
                nc.sync.dma_start(z_sb[:, 0:2, :], z_ap[:, 0:2, :])
                nc.scalar.dma_start(z_sb[:, 2:4, :], z_ap[:, 2:4, :])
                for hj in range(HJ):
                    if hj < 2:
                        nc.scalar.activation(zr_sb[:, hj, :], z_sb[:, hj, :],
                                             AF.Relu,
                                             bias=bf1_sb[:, hj:hj + 1])
                    else:
                        nc.vector.tensor_scalar(zr_sb[:, hj, :],
                                                z_sb[:, hj, :],
                                                bf1_sb[:, hj:hj + 1], 0.0,
                                                ALU.add, ALU.max)

                ps_o = ps_mix.tile([BPC, N_OUT], f32, tag="psm", name="ps_o")
                for hj in range(HJ):
                    nc.tensor.matmul(ps_o[:], zr_sb[:, hj, :],
                                     wf2_sb[:, hj, :],
                                     start=(hj == 0), stop=False)
                nc.tensor.matmul(ps_o[:], ones[0:1, 0:BPC], bf2_sb[:],
                                 start=False, stop=True)

                mx = small.tile([BPC, 1], f32)
                nc.vector.tensor_reduce(mx[:], ps_o[:], axis=AX.X, op=ALU.max,
                                        negate=True)
                t = small.tile([BPC, N_OUT], f32)
                nc.vector.tensor_scalar(t[:], ps_o[:], mx[0:BPC, 0:1], None,
                                        ALU.add)
                ex = small.tile([BPC, N_OUT], f32)
                nc.scalar.activation(ex[:], t[:], AF.Exp)
                sm = small.tile([BPC, 1], f32)
                nc.vector.tensor_reduce(sm[:], ex[:], axis=AX.X, op=ALU.add)
                rc = small.tile([BPC, 1], f32)
                nc.vector.reciprocal(rc[:], sm[:])
                ob = small.tile([BPC, N_OUT], f32)
                nc.vector.tensor_scalar(ob[:], ex[:], rc[0:BPC, 0:1], None,
                                        ALU.mult)
                nc.sync.dma_start(out_d[:], ob[:])

    _install_wait_splitter(nc)
    return nc


_NC_CACHE = None


def _get_program():
    global _NC_CACHE
    if _NC_CACHE is None:
        _NC_CACHE = _build_program()
    return _NC_CACHE


# ---------------------------------------------------------------------------
def _prep_inputs(x, edge_row, edge_col, edge_val, W1, b1, W2, b2,
                 Wf1, bf1, Wf2, bf2):
    import ml_dtypes
    f = np.float32
    bf = ml_dtypes.bfloat16
    A = np.zeros((N, N), f)
    np.add.at(A, (np.asarray(edge_row), np.asarray(edge_col)),
              np.asarray(edge_val, f))
    AT = np.ascontiguousarray(A.T)                                  # [m, n]
    at = np.ascontiguousarray(
        AT.reshape(KN, P, N).transpose(1, 0, 2).reshape(P, KN * N)).astype(bf)

    XT = np.ascontiguousarray(np.asarray(x, f)[:, :, 0].T)          # [N, B]
    xt = np.ascontiguousarray(
        XT.reshape(KN, P, B).transpose(1, 0, 2).reshape(P, KN * B)).astype(bf)

    W1 = np.asarray(W1, f); b1 = np.asarray(b1, f)
    W2 = np.asarray(W2, f); b2 = np.asarray(b2, f)
    Wf1 = np.asarray(Wf1, f); bf1 = np.asarray(bf1, f)
    Wf2 = np.asarray(Wf2, f); bf2 = np.asarray(bf2, f)

    # mix weight: lhsT[(ng,c),(ng',c')] = delta(ng,ng') * W2[c,c']
    w2k = np.kron(np.eye(NG, dtype=f), W2).astype(bf)               # [128,128]
    b2k = np.tile(b2, NG).reshape(128, 1).astype(f)

    # FC1: core k's K-chunk s holds flat rows (n=112k+ng*28+s)*C + c' at
    # partition p = ng*C + c'; rows for pad nodes (n >= 784) are zero.
    Wf1_pad = np.zeros((NPAD, C, H), f)
    Wf1_pad[:N] = Wf1.reshape(N, C, H)

    bf1_l = np.ascontiguousarray(bf1.reshape(HJ, 128).T)            # [128, HJ]
    wf2_l = np.ascontiguousarray(
        Wf2.reshape(HJ, 128, N_OUT).transpose(1, 0, 2).reshape(
            128, HJ * N_OUT)).astype(bf)
    bf2_l = bf2.reshape(1, N_OUT).astype(bf)

    in_maps = []
    for k in range(NCORE):
        wb = np.concatenate([W1[0, k * CPC:(k + 1) * CPC],
                             b1[k * CPC:(k + 1) * CPC]]).reshape(1, 2 * CPC)
        wb128 = np.ascontiguousarray(np.tile(wb, (128, 1)).astype(f))
        # [NG, NS, C, H] -> chunk s, partition (ng, c')
        wk = Wf1_pad[k * P:(k + 1) * P].reshape(NG, NS, C, H)
        wf1_l = np.ascontiguousarray(
            wk.transpose(1, 0, 2, 3).reshape(NS * 128, H)).astype(bf)
        in_maps.append({
            "at": at, "xt": xt, "wf1": wf1_l,
            "wb": wb128,
            "w2k": w2k, "b2k": b2k,
            "bf1": bf1_l, "wf2": wf2_l, "bf2": bf2_l,
        })
    return in_maps


def kernel(x, edge_row, edge_col, edge_val, W1, b1, W2, b2,
           Wf1, bf1, Wf2, bf2, **kw):
    nc = _get_program()
    in_maps = _prep_inputs(x, edge_row, edge_col, edge_val, W1, b1, W2, b2,
                           Wf1, bf1, Wf2, bf2)
    res = run_bass_kernel_spmd(nc, in_maps, list(range(NCORE)), **kw)
    out = np.concatenate([res.results[k]["out"] for k in range(NCORE)], axis=0)
    if kw.get("trace"):
        kernel.last_exec_time_ns = res.exec_time_ns
    return out.astype(np.float32)


# revision 28
# speedup vs baseline: 1.1731x; 1.1731x over previous
"""GNN message-passing net on 8 Trainium2 cores.

Reference: x:[256,784,1] -> h1 = elu(spmm(x)@W1+b1) -> h2 = elu(spmm(h1)@W2+b2)
-> flat[B, N*C] -> relu(flat@Wf1+bf1) -> softmax(z@Wf2+bf2).

Strategy (all matmul operands bf16, fp32 PSUM accumulation):
  * Densify the sparse filter A (784x784, ~1% nz) on the host; spmm becomes
    dense matmuls on the PE array.
  * F=1 makes conv1 an outer product: out1 = A @ X^T [784,256] shared by all
    channels; h1_c = elu(W1[c]*out1+b1[c]) via the exact identity
    elu(t) = max(t, min(exp(t),1)-1): one ACT Exp pass + three DVE ops.
  * Conv2 spmm channel-sharded: core k computes out2_c = A @ h1_c for
    channels 4k..4k+3, in two batch halves with kc-OUTER accumulation (6
    PSUM accumulators + one kc-inner chain) so the first half consumes h1
    chunks as the conv1 elu pipeline produces them instead of waiting for
    all of h1.
  * One AllToAll reshards channel->node: core j gets all 32 pre-mix
    channels for nodes [112j, 112j+112) (core 7 gets zero-padded nodes
    784..895), packed [(ng,c), s, b].  Packs are spread across three DMA
    queues as each node block completes; the unpack is s-range-major so
    the W2 mix starts before the full payload has landed.
  * W2 channel mix as a 128x128 stationary kron(I4,W2) matmul; +b2, elu.
  * W2 mix runs as 4-node-group chains in two-bank PSUM tiles so the elu
    chain latency amortizes over 1024-wide tiles.
  * FC1 stays K-sharded: core k holds Wf1 rows for its nodes, 28 K-chunks x
    4 h-chunks, free=256.  z^T partials [512,256] are ReduceScattered in
    bf16 (halves the collective payload); each core then does +bf1, relu,
    FC2 (+bf2 via a ones-row matmul) and softmax for its 32-batch block.
  Note: each matmul accumulation chain needs its own PSUM bank — two
  start/stop chains sharing a bank corrupt each other.
"""
import json

import numpy as np

import concourse.bass as bass
import concourse.mybir as mybir
import concourse.tile as tile
from concourse.bass_utils import run_bass_kernel_spmd

B, N, F, E = 256, 784, 1, 6272
C, H, N_OUT = 32, 512, 10
NCORE = 8
CPC = C // NCORE      # 4 channels per core in conv2
P = 112               # 784 = 7 * 112
KN = N // P           # 7 node chunks
NPAD = P * NCORE      # 896 padded nodes for the node reshard
NG = 4                # node groups packed into partitions for the mix
NS = P // NG          # 28 nodes per group per core
BPC = B // NCORE      # 32 batch rows per core
HJ = H // 128         # 4 h chunks
BH = 2                # batch halves for the pipelined a2a
BB = B // BH          # 128 batch rows per half

f32 = mybir.dt.float32
bf16 = mybir.dt.bfloat16
AF = mybir.ActivationFunctionType
ALU = mybir.AluOpType
AX = mybir.AxisListType

GROUPS = [list(range(NCORE))]


# ---------------------------------------------------------------------------
# BIR post-pass: this walrus build rejects instructions with >1 sync-wait;
# split extras onto standalone EventSemaphore instructions (same engine,
# inserted just before, so the engine stream stalls identically).
def _split_waits(bir: dict, max_waits: int = 1) -> dict:
    n = [0]
    for fn in bir.get("functions", []):
        for blk in fn.get("blocks", []):
            out = []
            for ins in blk.get("instructions", []):
                si = ins.get("sync_info") or {}
                waits = si.get("on_wait") or []
                if len(waits) > max_waits:
                    for w in waits[max_waits:]:
                        n[0] += 1
                        out.append({
                            "name": f"I-waitsplit-{n[0]}",
                            "opcode": "EventSemaphore",
                            "engine": ins["engine"],
                            "ins": [], "outs": [],
                            **({"debug": ins["debug"]} if "debug" in ins else {}),
                            "sync_info": {"on_update": [], "on_wait": [w]},
                        })
                    si = dict(si)
                    si["on_wait"] = waits[:max_waits]
                    ins = dict(ins)
                    ins["sync_info"] = si
                out.append(ins)
            blk["instructions"] = out
    return bir


def _install_wait_splitter(nc):
    orig = nc.to_json_bytes
    nc.to_json_bytes = lambda: json.dumps(_split_waits(json.loads(orig()))).encode()


# ---------------------------------------------------------------------------
def _build_program():
    nc = bass.Bass(num_devices=NCORE)

    at_d = nc.dram_tensor("at", [P, KN * N], bf16, kind="ExternalInput")
    xt_d = nc.dram_tensor("xt", [P, KN * B], bf16, kind="ExternalInput")
    wf1_d = nc.dram_tensor("wf1", [NS * 128, H], bf16, kind="ExternalInput")
    wb_d = nc.dram_tensor("wb", [128, 2 * CPC], f32, kind="ExternalInput")
    w2k_d = nc.dram_tensor("w2k", [128, 128], bf16, kind="ExternalInput")
    b2k_d = nc.dram_tensor("b2k", [128, 1], f32, kind="ExternalInput")
    bf1_d = nc.dram_tensor("bf1", [128, HJ], f32, kind="ExternalInput")
    wf2_d = nc.dram_tensor("wf2", [128, HJ * N_OUT], bf16, kind="ExternalInput")
    bf2_d = nc.dram_tensor("bf2", [1, N_OUT], bf16, kind="ExternalInput")
    out_d = nc.dram_tensor("out", [BPC, N_OUT], f32, kind="ExternalOutput")

    with tile.TileContext(nc) as tc:
        with (
            tc.tile_pool(name="big", bufs=1) as big,
            tc.tile_pool(name="small", bufs=1) as small,
            tc.tile_pool(name="ework", bufs=2) as ework,
            tc.tile_pool(name="twork", bufs=2) as twork,
            tc.tile_pool(name="dram", bufs=1, space="DRAM") as dram,
        ):
            # ---- resident inputs ------------------------------------------
            at_sb = big.tile([P, KN, N], bf16)
            xt_sb = big.tile([P, KN, B], bf16)
            wf1_sb = big.tile([128, NS, H], bf16)
            wband = small.tile([128, 2 * CPC], f32)
            w2k_sb = small.tile([128, 128], bf16)
            b2k_sb = small.tile([128, 1], f32)
            bf1_sb = small.tile([128, HJ], f32)
            wf2_sb = small.tile([128, HJ, N_OUT], bf16)
            bf2_sb = small.tile([1, N_OUT], bf16)
            ones = small.tile([1, 128], bf16)
            zpad = small.tile([P, CPC, B], bf16)

            nc.sync.dma_start(wband[:], wb_d[:])
            at_ap = at_d[:].rearrange("p (k n) -> p k n", k=KN)
            xt_ap = xt_d[:].rearrange("p (k b) -> p k b", k=KN)
            for kc in range(KN):
                nc.sync.dma_start(xt_sb[:, kc, :], xt_ap[:, kc, :])
                nc.sync.dma_start(at_sb[:, kc, :], at_ap[:, kc, :])
            nc.sync.dma_start(w2k_sb[:], w2k_d[:])
            nc.sync.dma_start(b2k_sb[:], b2k_d[:])
            nc.sync.dma_start(bf1_sb[:], bf1_d[:])
            nc.sync.dma_start(wf2_sb[:], wf2_d[:].rearrange("p (j o) -> p j o",
                                                            j=HJ))
            nc.sync.dma_start(bf2_sb[:], bf2_d[:])
            ab_sb = small.tile([NCORE, 2], f32)
            nc.vector.memset(ab_sb[:], 0.0)
            nc.vector.memset(ones[:], 1.0)
            nc.vector.memset(zpad[:], 0.0)
            ab_d = dram.tile([NCORE, 2], f32)
            nc.sync.dma_start(ab_d[:], ab_sb[:])
            nc.sync.dma_start(wf1_sb[:],
                              wf1_d[:].rearrange("(s p) h -> p s h", p=128))
            # warm-up collective with no consumers: pays the first-collective
            # setup (~12us) and any launch skew on the CC engine while the
            # compute engines run conv1, so the real a2a starts promptly.
            # Same kind as the real collective so any AllToAll-specific ring
            # setup is prepaid too.
            ab_o = dram.tile([NCORE, 2], f32)
            nc.gpsimd.collective_compute(
                "AllToAll", ALU.bypass, replica_groups=GROUPS,
                ins=[ab_d.opt()], outs=[ab_o.opt()],
            )


            h1_sb = big.tile([P, CPC, KN, B], bf16)
            out2_sb = big.tile([P, CPC, KN, B], bf16)
            r_sb = big.tile([128, NS, B], bf16)
            h2_sb = big.tile([128, NS, B], bf16)

            a2a_in = dram.tile([NCORE, CPC * P, B], bf16)
            a2a_out = dram.tile([NCORE, CPC * P, B], bf16)
            # pad-block pack has no compute deps: send it now, ~60us before
            # the a2a doorbell, so it never gates the collective.
            zp_dst = a2a_in[KN].rearrange("(cl p) b -> p cl b", cl=CPC)
            nc.gpsimd.dma_start(zp_dst, zpad[:])

            with (
                tc.tile_pool(name="ps1", bufs=2, space="PSUM") as ps1,
                tc.tile_pool(name="ps2", bufs=KN - 1, space="PSUM") as ps2,
            ):
                # ---- conv1 + elu ------------------------------------------
                for mc in range(KN):
                    o1 = ps1.tile([P, B], f32, tag="ps1", name="o1")
                    for kc in range(KN):
                        nc.tensor.matmul(
                            o1[:],
                            at_sb[:, kc, mc * P:(mc + 1) * P],
                            xt_sb[:, kc, :],
                            start=(kc == 0), stop=(kc == KN - 1),
                        )
                    e4 = ework.tile([P, CPC, B], bf16, name="e4")
                    t4 = twork.tile([P, CPC, B], bf16, name="t4")
                    for c in range(CPC):
                        nc.scalar.activation(
                            e4[:, c, :], o1[:], AF.Exp,
                            bias=wband[0:P, CPC + c:CPC + c + 1],
                            scale=wband[0:P, c:c + 1])
                    for c in range(CPC):
                        if c == 3:
                            nc.scalar.activation(
                                t4[:, c, :], o1[:], AF.Identity,
                                bias=wband[0:P, CPC + c:CPC + c + 1],
                                scale=wband[0:P, c:c + 1])
                        else:
                            nc.vector.tensor_scalar(
                                t4[:, c, :], o1[:],
                                wband[0:P, c:c + 1],
                                wband[0:P, CPC + c:CPC + c + 1],
                                ALU.mult, ALU.add)
                    nc.vector.tensor_scalar(e4[:], e4[:], 1.0, -1.0,
                                            ALU.min, ALU.add)
                    nc.vector.tensor_tensor(h1_sb[:, :, mc, :], t4[:], e4[:],
                                            ALU.max)

                # ---- conv2 spmm: kc-outer per batch half (overlaps the
                # conv1 elu pipeline), 7 accumulators, single a2a ----------
                pack_engines = [nc.gpsimd, nc.sync, nc.scalar]
                for bh in range(BH):
                    bs = bh * BB
                    o2 = [ps2.tile([P, CPC, BB], f32, tag="ps2", name="o2")
                          for mo in range(KN - 1)]
                    for kc in range(KN):
                        for mo in range(KN - 1):
                            nc.tensor.matmul(
                                o2[mo][:],
                                at_sb[:, kc, mo * P:(mo + 1) * P],
                                h1_sb[:, :, kc, bs:bs + BB],
                                start=(kc == 0), stop=(kc == KN - 1),
                            )
                    for mo in range(KN - 1):
                        if mo % 2 == 0:
                            nc.scalar.copy(out2_sb[:, :, mo, bs:bs + BB],
                                           o2[mo][:])
                        else:
                            nc.vector.tensor_copy(
                                out2_sb[:, :, mo, bs:bs + BB], o2[mo][:])
                        dstp = a2a_in[mo].rearrange(
                            "(cl p) b -> p cl b", cl=CPC)[:, :, bs:bs + BB]
                        pack_engines[mo % 3].dma_start(
                            dstp, out2_sb[:, :, mo, bs:bs + BB])
                    # mo = KN-1: plain kc-inner chain (no spare PSUM bank)
                    o2l = ps2.tile([P, CPC, BB], f32, tag="ps2", name="o2")
                    for kc in range(KN):
                        nc.tensor.matmul(
                            o2l[:],
                            at_sb[:, kc, (KN - 1) * P:KN * P],
                            h1_sb[:, :, kc, bs:bs + BB],
                            start=(kc == 0), stop=(kc == KN - 1),
                        )
                    nc.scalar.copy(out2_sb[:, :, KN - 1, bs:bs + BB], o2l[:])
                    dstp = a2a_in[KN - 1].rearrange(
                        "(cl p) b -> p cl b", cl=CPC)[:, :, bs:bs + BB]
                    pack_engines[(KN - 1) % 3].dma_start(
                        dstp, out2_sb[:, :, KN - 1, bs:bs + BB])
                nc.gpsimd.collective_compute(
                    "AllToAll", ALU.bypass, replica_groups=GROUPS,
                    ins=[a2a_in.opt()], outs=[a2a_out.opt()],
                )
                # unpack s-range-major so the W2 mix can start before the
                # whole payload has landed; spread across DMA queues.
                ap = a2a_out[:].rearrange(
                    "k (cl g s) b -> g (k cl) s b", cl=CPC, g=NG)
                unpack_engines = [nc.sync, nc.scalar, nc.gpsimd]
                ui = 0
                for s0 in range(0, NS, 7):
                    for ng in range(NG):
                        unpack_engines[ui % 3].dma_start(
                            r_sb[ng * C:(ng + 1) * C, s0:s0 + 7, :],
                            ap[ng, :, s0:s0 + 7, :])
                        ui += 1

            with (
                tc.tile_pool(name="ps_mix", bufs=2, space="PSUM") as ps_mix,
                tc.tile_pool(name="ps_fc1", bufs=HJ, space="PSUM") as ps_fc1,
            ):
                # ---- W2 mix + b2 + elu -> h2 (bf16) -----------------------
                zps = [ps_fc1.tile([128, B], f32, tag="psfc1", name="zp")
                       for h in range(HJ)]
                for s0 in range(0, NS, 4):
                    pm = ps_mix.tile([128, 4, B], f32, tag="psm", name="pm")
                    nc.tensor.matmul(pm[:, 0:2, :], w2k_sb[:],
                                     r_sb[:, s0:s0 + 2, :])
                    nc.tensor.matmul(pm[:, 2:4, :], w2k_sb[:],
                                     r_sb[:, s0 + 2:s0 + 4, :])
                    em = ework.tile([128, 4, B], bf16, name="em")
                    tm = twork.tile([128, 4, B], bf16, name="tm")
                    nc.scalar.activation(em[:], pm[:], AF.Exp,
                                         bias=b2k_sb[:, 0:1])
                    if (s0 // 4) % 2 == 0:
                        nc.scalar.activation(tm[:], pm[:], AF.Identity,
                                             bias=b2k_sb[:, 0:1])
                    else:
                        nc.vector.tensor_scalar(tm[:], pm[:], b2k_sb[:, 0:1],
                                                None, ALU.add)
                    nc.vector.tensor_scalar(em[:], em[:], 1.0, -1.0,
                                            ALU.min, ALU.add)
                    nc.vector.tensor_tensor(h2_sb[:, s0:s0 + 4, :],
                                            tm[:], em[:], ALU.max)
                    # FC1 s-groups trail the mix by two chains so their
                    # h2 waits are already satisfied at issue time (a lag of
                    # zero head-of-line-blocks the next chain's matmuls).
                    if s0 >= 8:
                        for s in range(s0 - 8, s0 - 4):
                            for hj in range(HJ):
                                nc.tensor.matmul(
                                    zps[hj][:],
                                    wf1_sb[:, s, hj * 128:(hj + 1) * 128],
                                    h2_sb[:, s, :],
                                    start=(s == 0), stop=(s == NS - 1),
                                )
                for s in range(NS - 8, NS):
                    for hj in range(HJ):
                        nc.tensor.matmul(
                            zps[hj][:],
                            wf1_sb[:, s, hj * 128:(hj + 1) * 128],
                            h2_sb[:, s, :],
                            start=(s == 0), stop=(s == NS - 1),
                        )

                # ---- ReduceScatter z partials -----------------------------
                zsb = small.tile([128, HJ, B], bf16)
                for hj in range(HJ):
                    if hj % 2 == 0:
                        nc.scalar.copy(zsb[:, hj, :], zps[hj][:])
                    else:
                        nc.vector.tensor_copy(zsb[:, hj, :], zps[hj][:])
                rs_in = dram.tile([NCORE, H, BPC], bf16)
                rs_out = dram.tile([H, BPC], bf16)
                rdst = rs_in[:].rearrange("j h b -> h j b")
                rs_engines = [nc.sync, nc.scalar, nc.gpsimd, nc.sync]
                for hj in range(HJ):
                    rs_engines[hj].dma_start(
                        rdst[hj * 128:(hj + 1) * 128],
                        zsb[:, hj, :].rearrange("h (j b) -> h j b", j=NCORE),
                    )
                nc.gpsimd.collective_compute(
                    "ReduceScatter", ALU.add, replica_groups=GROUPS,
                    ins=[rs_in.opt()], outs=[rs_out.opt()],
                )

                # ---- +bf1, relu, FC2, +bf2, softmax -----------------------
                z_sb = small.tile([128, HJ, BPC], bf16)
                zr_sb = small.tile([128, HJ, BPC], bf16)
                z_ap = rs_out[:].rearrange("(hj p) b -> p hj b", p=128)
                nc.sync.dma_start(z_sb[:, 0:2, :], z_ap[:, 0:2, :])
                nc.scalar.dma_start(z_sb[:, 2:4, :], z_ap[:, 2:4, :])
                for hj in range(HJ):
                    if hj < 2:
                        nc.scalar.activation(zr_sb[:, hj, :], z_sb[:, hj, :],
                                             AF.Relu,
                                             bias=bf1_sb[:, hj:hj + 1])
                    else:
                        nc.vector.tensor_scalar(zr_sb[:, hj, :],
                                                z_sb[:, hj, :],
                                                bf1_sb[:, hj:hj + 1], 0.0,
                                                ALU.add, ALU.max)

                ps_o = ps_mix.tile([BPC, N_OUT], f32, tag="psm", name="ps_o")
                for hj in range(HJ):
                    nc.tensor.matmul(ps_o[:], zr_sb[:, hj, :],
                                     wf2_sb[:, hj, :],
                                     start=(hj == 0), stop=False)
                nc.tensor.matmul(ps_o[:], ones[0:1, 0:BPC], bf2_sb[:],
                                 start=False, stop=True)

                mx = small.tile([BPC, 1], f32)
                nc.vector.tensor_reduce(mx[:], ps_o[:], axis=AX.X, op=ALU.max,
                                        negate=True)
                t = small.tile([BPC, N_OUT], f32)
                nc.vector.tensor_scalar(t[:], ps_o[:], mx[0:BPC, 0:1], None,
                                        ALU.add)
                ex = small.tile([BPC, N_OUT], f32)
                nc.scalar.activation(ex[:], t[:], AF.Exp)
                sm = small.tile([BPC, 1], f32)
                nc.vector.tensor_reduce(sm[:], ex[:], axis=AX.X, op=ALU.add)
                rc = small.tile([BPC, 1], f32)
                nc.vector.reciprocal(rc[:], sm[:])
                ob = small.tile([BPC, N_OUT], f32)
                nc.vector.tensor_scalar(ob[:], ex[:], rc[0:BPC, 0:1], None,
                                        ALU.mult)
                nc.sync.dma_start(out_d[:], ob[:])

    _install_wait_splitter(nc)
    return nc


_NC_CACHE = None


def _get_program():
    global _NC_CACHE
    if _NC_CACHE is None:
        _NC_CACHE = _build_program()
    return _NC_CACHE


# ---------------------------------------------------------------------------
def _prep_inputs(x, edge_row, edge_col, edge_val, W1, b1, W2, b2,
                 Wf1, bf1, Wf2, bf2):
    import ml_dtypes
    f = np.float32
    bf = ml_dtypes.bfloat16
    A = np.zeros((N, N), f)
    np.add.at(A, (np.asarray(edge_row), np.asarray(edge_col)),
              np.asarray(edge_val, f))
    AT = np.ascontiguousarray(A.T)                                  # [m, n]
    at = np.ascontiguousarray(
        AT.reshape(KN, P, N).transpose(1, 0, 2).reshape(P, KN * N)).astype(bf)

    XT = np.ascontiguousarray(np.asarray(x, f)[:, :, 0].T)          # [N, B]
    xt = np.ascontiguousarray(
        XT.reshape(KN, P, B).transpose(1, 0, 2).reshape(P, KN * B)).astype(bf)

    W1 = np.asarray(W1, f); b1 = np.asarray(b1, f)
    W2 = np.asarray(W2, f); b2 = np.asarray(b2, f)
    Wf1 = np.asarray(Wf1, f); bf1 = np.asarray(bf1, f)
    Wf2 = np.asarray(Wf2, f); bf2 = np.asarray(bf2, f)

    # mix weight: lhsT[(ng,c),(ng',c')] = delta(ng,ng') * W2[c,c']
    w2k = np.kron(np.eye(NG, dtype=f), W2).astype(bf)               # [128,128]
    b2k = np.tile(b2, NG).reshape(128, 1).astype(f)

    # FC1: core k's K-chunk s holds flat rows (n=112k+ng*28+s)*C + c' at
    # partition p = ng*C + c'; rows for pad nodes (n >= 784) are zero.
    Wf1_pad = np.zeros((NPAD, C, H), f)
    Wf1_pad[:N] = Wf1.reshape(N, C, H)

    bf1_l = np.ascontiguousarray(bf1.reshape(HJ, 128).T)            # [128, HJ]
    wf2_l = np.ascontiguousarray(
        Wf2.reshape(HJ, 128, N_OUT).transpose(1, 0, 2).reshape(
            128, HJ * N_OUT)).astype(bf)
    bf2_l = bf2.reshape(1, N_OUT).astype(bf)

    in_maps = []
    for k in range(NCORE):
        wb = np.concatenate([W1[0, k * CPC:(k + 1) * CPC],
                             b1[k * CPC:(k + 1) * CPC]]).reshape(1, 2 * CPC)
        wb128 = np.ascontiguousarray(np.tile(wb, (128, 1)).astype(f))
        # [NG, NS, C, H] -> chunk s, partition (ng, c')
        wk = Wf1_pad[k * P:(k + 1) * P].reshape(NG, NS, C, H)
        wf1_l = np.ascontiguousarray(
            wk.transpose(1, 0, 2, 3).reshape(NS * 128, H)).astype(bf)
        in_maps.append({
            "at": at, "xt": xt, "wf1": wf1_l,
            "wb": wb128,
            "w2k": w2k, "b2k": b2k,
            "bf1": bf1_l, "wf2": wf2_l, "bf2": bf2_l,
        })
    return in_maps


def kernel(x, edge_row, edge_col, edge_val, W1, b1, W2, b2,
           Wf1, bf1, Wf2, bf2, **kw):
    nc = _get_program()
    in_maps = _prep_inputs(x, edge_row, edge_col, edge_val, W1, b1, W2, b2,
                           Wf1, bf1, Wf2, bf2)
    res = run_bass_kernel_spmd(nc, in_maps, list(range(NCORE)), **kw)
    out = np.concatenate([res.results[k]["out"] for k in range(NCORE)], axis=0)
    if kw.get("trace"):
        kernel.last_exec_time_ns = res.exec_time_ns
    return out.astype(np.float32)
